# revision 7
# baseline (speedup 1.0000x reference)
"""Trainium2 Bass kernel for -mean(antonymy_score > synonymy_score).

Strategy: pure data-parallel over 8 NeuronCores. Each core receives a
contiguous 1/8 slice of the batch. On the host, antonymy/synonymy are
interleaved into one "pairs" tensor [n_chunks, 128, 2, chunk_fd] so that
each 2 MiB chunk arrives in a single contiguous DMA (one semaphore wait -
the DVE TensorScalarPtr instruction only supports a single sync wait).
Each chunk is consumed by one fused scalar_tensor_tensor instruction:
mask = (ant bypass 0) is_gt syn, with accum_out giving the per-partition
free-dim sum in the same pass. The [128, n_chunks] partial counts are
DMA'd back and the host computes -total/B (exact: integer-valued fp32).
S1_out is unused by the computation - it only fixes the batch size.
"""

import numpy as np

import concourse.bass as bass
import concourse.mybir as mybir
import concourse.tile as tile
import concourse.tile_sem_assignment as _tsa
from concourse.bass_utils import run_bass_kernel_spmd

# The bass2jax/walrus codegen path used here allows very few sync-wait
# commands per instruction (1 for DMA/compute, and the kernel-tail Drain
# failed with 6). Collapse all HWDGE DMA completions onto a single
# semaphore lane: every DMA below issues on the one SP HWDGE ring, which
# executes FIFO, so cumulative waits on a single shared lane are safe.
_tsa.NUM_HWDGE_SEMS = 1

B = 8388608
N_CORES = 8
PER_CORE = B // N_CORES  # 1048576
P = 128
CHUNK_FD = 2048  # per array; one pair-chunk DMA = 128*2*2048*4B = 2 MiB
N_CHUNKS = PER_CORE // (P * CHUNK_FD)  # 4

_NC = None


def build_nc():
    nc = bass.Bass()
    pairs = nc.dram_tensor(
        "pairs", [N_CHUNKS, P, 2, CHUNK_FD], mybir.dt.float32, kind="ExternalInput"
    )
    out = nc.dram_tensor(
        "out", [P, N_CHUNKS], mybir.dt.float32, kind="ExternalOutput"
    )

    with tile.TileContext(nc) as tc:
        with (
            tc.tile_pool(name="io", bufs=N_CHUNKS) as io,
            tc.tile_pool(name="acc", bufs=1) as accp,
        ):
            partials = accp.tile([P, N_CHUNKS], mybir.dt.float32)
            for i in range(N_CHUNKS):
                pair_t = io.tile([P, 2, CHUNK_FD], mybir.dt.float32, tag="pair")
                mask_t = io.tile([P, CHUNK_FD], mybir.dt.float32, tag="mask")
                nc.sync.dma_start(pair_t[:], pairs[i])
                # mask = (ant bypass 0.0) is_gt syn  -> 1.0/0.0
                # partials[:, i] = free-dim sum of mask (same instruction)
                nc.vector.scalar_tensor_tensor(
                    out=mask_t[:],
                    in0=pair_t[:, 0, :],
                    scalar=0.0,
                    in1=pair_t[:, 1, :],
                    op0=mybir.AluOpType.bypass,
                    op1=mybir.AluOpType.is_gt,
                    accum_out=partials[:, i : i + 1],
                )
            nc.sync.dma_start(out[:], partials[:])
    _prune_redundant_waits(nc)
    return nc


def _prune_redundant_waits(nc):
    """The walrus codegen path here supports only 1 sync wait on DMA/compute
    instructions and <3 on Drain. Two emitted waits are transitively
    redundant in this kernel and can be dropped:

    - The final store ("out" DMA) gets [DVE>=4, DMAHW0>=64]. Its only data
      dependency is the partials tile written by the DVE STTs; the lane
      wait merely reflects FIFO position on the shared lane (the HWDGE
      ring executes FIFO regardless, and the store touches no pair tiles).
      Keep only the DVE wait.
    - The kernel-tail Drain gets [DVE>=4, DMAHW0>=80]. The lane hitting 80
      requires the out-store to have completed, which in turn required
      DVE>=4 before it issued. Keep only the lane wait.
    """
    for bb in nc.main_func.blocks:
        for ins in bb.instructions:
            si = getattr(ins, "sync_info", None)
            if si is None or len(si.on_wait) < 2:
                continue
            tname = type(ins).__name__
            if tname == "InstDMACopy":
                outs = getattr(ins, "outs", [])
                assert outs and getattr(outs[0], "memref", "") == "out", (
                    f"unexpected multi-wait DMA {ins.name}"
                )
                keep = [w for w in si.on_wait if w.ant_name.startswith("DVE")]
            elif tname == "InstDrain":
                keep = [w for w in si.on_wait if w.ant_name.startswith("DMAHW")]
            else:
                raise AssertionError(
                    f"unexpected multi-wait instruction {ins.name} ({tname})"
                )
            assert len(keep) == 1, (ins.name, [w.ant_name for w in si.on_wait])
            si.on_wait[:] = keep


def _make_pairs(synonymy_score, antonymy_score):
    syn = np.asarray(synonymy_score, dtype=np.float32).reshape(
        N_CORES, N_CHUNKS, P, CHUNK_FD
    )
    ant = np.asarray(antonymy_score, dtype=np.float32).reshape(
        N_CORES, N_CHUNKS, P, CHUNK_FD
    )
    # [C, N, P, 2, FD]; index 0 = ant (in0), index 1 = syn (in1)
    return np.ascontiguousarray(np.stack([ant, syn], axis=3))


def run(inputs, trace=False, trace_cores=None):
    """Run the SPMD kernel on 8 cores. Returns (result_scalar, BassKernelResults)."""
    global _NC
    if _NC is None:
        _NC = build_nc()

    pairs = _make_pairs(inputs["synonymy_score"], inputs["antonymy_score"])
    in_maps = [{"pairs": pairs[c]} for c in range(N_CORES)]
    bkr = run_bass_kernel_spmd(
        _NC,
        in_maps,
        list(range(N_CORES)),
        trace=trace,
        trace_cores=trace_cores,
    )
    total = sum(
        np.asarray(r["out"], dtype=np.float64).sum() for r in bkr.results
    )
    result = np.float32(-(total / B))
    return result, bkr


def kernel(S1_out, synonymy_score, antonymy_score):
    result, _ = run(
        {"synonymy_score": synonymy_score, "antonymy_score": antonymy_score}
    )
    return result


# revision 11
# speedup vs baseline: 1.1719x; 1.1719x over previous
"""Trainium2 Bass kernel for -mean(antonymy_score > synonymy_score).

Strategy: pure data-parallel over 8 NeuronCores. Each core receives a
contiguous 1/8 slice of the batch. On the host, antonymy/synonymy are
interleaved into one "pairs" tensor [n_chunks, 128, 2, chunk_fd] so each
1 MiB chunk arrives in a single contiguous DMA carrying one semaphore
(the walrus codegen path here allows only one sync wait per
instruction). Chunks alternate between the two HWDGE rings (SP + ACT) so
the two streams transfer concurrently and approach the ~358 GB/s
per-core HBM limit. Each chunk is consumed by one fused DVE
scalar_tensor_tensor: mask = (ant bypass 0) is_gt syn, with accum_out
producing the per-partition free-dim sum in the same pass. The
[128, n_chunks] partial counts are DMA'd back and the host computes
-total/B (exact: integer-valued fp32 counts). S1_out is unused by the
computation - it only fixes the batch size.

Raw Bass (no TileContext) keeps the pre/postamble to a single
all-engine barrier instead of Tile's ~10us of drains + EVSEM
butterflies.
"""

import numpy as np

import concourse.bass as bass
import concourse.mybir as mybir
from concourse.bass_utils import run_bass_kernel_spmd

B = 8388608
N_CORES = 8
PER_CORE = B // N_CORES  # 1048576
P = 128
CHUNK_FD = 1024  # per array; one pair-chunk DMA = 128*2*1024*4B = 1 MiB
N_CHUNKS = PER_CORE // (P * CHUNK_FD)  # 8

F32 = mybir.dt.float32

_NC = None


def build_nc():
    nc = bass.Bass()
    pairs = nc.dram_tensor(
        "pairs", [N_CHUNKS, P, 2, CHUNK_FD], F32, kind="ExternalInput"
    )
    out = nc.dram_tensor("out", [P, N_CHUNKS], F32, kind="ExternalOutput")

    from contextlib import ExitStack

    with ExitStack() as ctx:
        pair_buf = ctx.enter_context(
            nc.sbuf_tensor("pair_buf", [P, N_CHUNKS, 2, CHUNK_FD], F32)
        )
        mask_buf = ctx.enter_context(
            nc.sbuf_tensor("mask_buf", [P, N_CHUNKS, CHUNK_FD], F32)
        )
        partials = ctx.enter_context(nc.sbuf_tensor("partials", [P, N_CHUNKS], F32))
        # One semaphore per chunk DMA: two in-flight DMAs sharing a sem can
        # interleave their 16 per-SDMA-engine increments, so a cumulative
        # wait could fire with the earlier chunk still incomplete.
        chunk_sems = [
            ctx.enter_context(nc.semaphore(f"chunk{k}")) for k in range(N_CHUNKS)
        ]
        dve_sem = ctx.enter_context(nc.semaphore("dve_sem"))
        out_sem = ctx.enter_context(nc.semaphore("out_sem"))
        block = ctx.enter_context(nc.Block())

        @block.sync
        def _(sync: bass.BassEngine):
            for k in range(0, N_CHUNKS, 2):
                sync.dma_start(pair_buf[:, k], pairs[k]).then_inc(chunk_sems[k], 16)
            sync.wait_ge(dve_sem, N_CHUNKS)
            sync.dma_start(out[:], partials[:]).then_inc(out_sem, 16)
            sync.wait_ge(out_sem, 16)

        @block.scalar
        def _(scalar: bass.BassEngine):
            for k in range(1, N_CHUNKS, 2):
                scalar.dma_start(pair_buf[:, k], pairs[k]).then_inc(
                    chunk_sems[k], 16
                )

        @block.vector
        def _(vector: bass.BassEngine):
            for k in range(N_CHUNKS):
                vector.wait_ge(chunk_sems[k], 16)
                # mask = (ant bypass 0.0) is_gt syn -> 1.0/0.0
                # partials[:, k] = free-dim sum of mask (same instruction)
                vector.scalar_tensor_tensor(
                    out=mask_buf[:, k],
                    in0=pair_buf[:, k, 0],
                    scalar=0.0,
                    in1=pair_buf[:, k, 1],
                    op0=mybir.AluOpType.bypass,
                    op1=mybir.AluOpType.is_gt,
                    accum_out=partials[:, k : k + 1],
                ).then_inc(dve_sem, 1)

    return nc


def _make_pairs(synonymy_score, antonymy_score):
    syn = np.asarray(synonymy_score, dtype=np.float32).reshape(
        N_CORES, N_CHUNKS, P, CHUNK_FD
    )
    ant = np.asarray(antonymy_score, dtype=np.float32).reshape(
        N_CORES, N_CHUNKS, P, CHUNK_FD
    )
    # [C, N, P, 2, FD]; index 0 = ant (in0), index 1 = syn (in1)
    return np.ascontiguousarray(np.stack([ant, syn], axis=3))


def run(inputs, trace=False, trace_cores=None):
    """Run the SPMD kernel on 8 cores. Returns (result_scalar, BassKernelResults)."""
    global _NC
    if _NC is None:
        _NC = build_nc()

    pairs = _make_pairs(inputs["synonymy_score"], inputs["antonymy_score"])
    in_maps = [{"pairs": pairs[c]} for c in range(N_CORES)]
    bkr = run_bass_kernel_spmd(
        _NC,
        in_maps,
        list(range(N_CORES)),
        trace=trace,
        trace_cores=trace_cores,
    )
    total = sum(
        np.asarray(r["out"], dtype=np.float64).sum() for r in bkr.results
    )
    result = np.float32(-(total / B))
    return result, bkr


def kernel(S1_out, synonymy_score, antonymy_score):
    result, _ = run(
        {"synonymy_score": synonymy_score, "antonymy_score": antonymy_score}
    )
    return result


# revision 12
# speedup vs baseline: 1.2949x; 1.1049x over previous
"""Trainium2 Bass kernel for -mean(antonymy_score > synonymy_score).

Strategy: pure data-parallel over 8 NeuronCores. Each core receives a
contiguous 1/8 slice of the batch. On the host, antonymy/synonymy are
interleaved into one "pairs" tensor [n_chunks, 128, 2, chunk_fd] so each
1 MiB chunk arrives in a single contiguous DMA carrying one semaphore
(the walrus codegen path here allows only one sync wait per
instruction). Chunks alternate between the two HWDGE rings (SP + ACT) so
the two streams transfer concurrently and approach the ~358 GB/s
per-core HBM limit. Each chunk is consumed by one fused DVE
scalar_tensor_tensor: mask = (ant bypass 0) is_gt syn, with accum_out
producing the per-partition free-dim sum in the same pass. The
[128, n_chunks] partial counts are DMA'd back and the host computes
-total/B (exact: integer-valued fp32 counts). S1_out is unused by the
computation - it only fixes the batch size.

Raw Bass (no TileContext) keeps the pre/postamble to a single
all-engine barrier instead of Tile's ~10us of drains + EVSEM
butterflies.
"""

import numpy as np

import concourse.bass as bass
import concourse.mybir as mybir
from concourse.bass_utils import run_bass_kernel_spmd

B = 8388608
N_CORES = 8
PER_CORE = B // N_CORES  # 1048576
P = 128
CHUNK_FD = 2048  # per array; one pair-chunk DMA = 128*2*2048*4B = 2 MiB
N_CHUNKS = PER_CORE // (P * CHUNK_FD)  # 4

F32 = mybir.dt.float32

_NC = None


def build_nc():
    nc = bass.Bass()
    pairs = nc.dram_tensor(
        "pairs", [N_CHUNKS, P, 2, CHUNK_FD], F32, kind="ExternalInput"
    )
    out = nc.dram_tensor("out", [P, N_CHUNKS], F32, kind="ExternalOutput")

    from contextlib import ExitStack

    with ExitStack() as ctx:
        pair_buf = ctx.enter_context(
            nc.sbuf_tensor("pair_buf", [P, N_CHUNKS, 2, CHUNK_FD], F32)
        )
        mask_buf = ctx.enter_context(
            nc.sbuf_tensor("mask_buf", [P, N_CHUNKS, CHUNK_FD], F32)
        )
        partials = ctx.enter_context(nc.sbuf_tensor("partials", [P, N_CHUNKS], F32))
        # One semaphore per chunk DMA: two in-flight DMAs sharing a sem can
        # interleave their 16 per-SDMA-engine increments, so a cumulative
        # wait could fire with the earlier chunk still incomplete.
        chunk_sems = [
            ctx.enter_context(nc.semaphore(f"chunk{k}")) for k in range(N_CHUNKS)
        ]
        dve_sem = ctx.enter_context(nc.semaphore("dve_sem"))
        out_sem = ctx.enter_context(nc.semaphore("out_sem"))
        block = ctx.enter_context(nc.Block())

        @block.sync
        def _(sync: bass.BassEngine):
            for k in range(0, N_CHUNKS, 2):
                sync.dma_start(pair_buf[:, k], pairs[k]).then_inc(chunk_sems[k], 16)
            sync.wait_ge(dve_sem, N_CHUNKS)
            sync.dma_start(out[:], partials[:]).then_inc(out_sem, 16)

        @block.scalar
        def _(scalar: bass.BassEngine):
            for k in range(1, N_CHUNKS, 2):
                scalar.dma_start(pair_buf[:, k], pairs[k]).then_inc(
                    chunk_sems[k], 16
                )

        @block.vector
        def _(vector: bass.BassEngine):
            for k in range(N_CHUNKS):
                vector.wait_ge(chunk_sems[k], 16)
                # mask = (ant bypass 0.0) is_gt syn -> 1.0/0.0
                # partials[:, k] = free-dim sum of mask (same instruction)
                vector.scalar_tensor_tensor(
                    out=mask_buf[:, k],
                    in0=pair_buf[:, k, 0],
                    scalar=0.0,
                    in1=pair_buf[:, k, 1],
                    op0=mybir.AluOpType.bypass,
                    op1=mybir.AluOpType.is_gt,
                    accum_out=partials[:, k : k + 1],
                ).then_inc(dve_sem, 1)

    return nc


def _make_pairs(synonymy_score, antonymy_score):
    syn = np.asarray(synonymy_score, dtype=np.float32).reshape(
        N_CORES, N_CHUNKS, P, CHUNK_FD
    )
    ant = np.asarray(antonymy_score, dtype=np.float32).reshape(
        N_CORES, N_CHUNKS, P, CHUNK_FD
    )
    # [C, N, P, 2, FD]; index 0 = ant (in0), index 1 = syn (in1)
    return np.ascontiguousarray(np.stack([ant, syn], axis=3))


def run(inputs, trace=False, trace_cores=None):
    """Run the SPMD kernel on 8 cores. Returns (result_scalar, BassKernelResults)."""
    global _NC
    if _NC is None:
        _NC = build_nc()

    pairs = _make_pairs(inputs["synonymy_score"], inputs["antonymy_score"])
    in_maps = [{"pairs": pairs[c]} for c in range(N_CORES)]
    bkr = run_bass_kernel_spmd(
        _NC,
        in_maps,
        list(range(N_CORES)),
        trace=trace,
        trace_cores=trace_cores,
    )
    total = sum(
        np.asarray(r["out"], dtype=np.float64).sum() for r in bkr.results
    )
    result = np.float32(-(total / B))
    return result, bkr


def kernel(S1_out, synonymy_score, antonymy_score):
    result, _ = run(
        {"synonymy_score": synonymy_score, "antonymy_score": antonymy_score}
    )
    return result


# revision 13
# speedup vs baseline: 1.3713x; 1.0590x over previous
"""Trainium2 Bass kernel for -mean(antonymy_score > synonymy_score).

Strategy: pure data-parallel over 8 NeuronCores. Each core receives a
contiguous 1/8 slice of the batch. On the host, antonymy/synonymy are
interleaved into one flat "pairs" tensor laid out as consecutive
[128, 2, fd_j] chunk blocks so each chunk arrives in a single contiguous
DMA carrying one semaphore (the walrus codegen path here allows only one
sync wait per instruction). Chunks alternate between the two HWDGE rings
(SP + ACT) so the two streams transfer concurrently (~341 GB/s observed,
~358 GB/s per-core HBM cap). Chunk sizes taper (2MB, 1MB, 0.5MB, 0.5MB
per ring) so the DVE work left after the last chunk lands is minimal.

Each chunk is consumed by one fused DVE scalar_tensor_tensor:
mask = (ant bypass 0) is_gt syn, with accum_out producing the
per-partition free-dim sum in the same pass. The [128, n_chunks] partial
counts are DMA'd back and the host computes -total/B (exact:
integer-valued fp32 counts). S1_out is unused by the computation - it
only fixes the batch size.

Raw Bass (no TileContext) keeps the program pre/postamble to a single
all-engine barrier instead of Tile's ~10us of drains + EVSEM
butterflies. One semaphore per chunk DMA: two in-flight DMAs sharing a
sem can interleave their 16 per-SDMA-engine increments, so a cumulative
wait could fire with the earlier chunk still incomplete.
"""

from contextlib import ExitStack

import numpy as np

import concourse.bass as bass
import concourse.mybir as mybir
from concourse.bass_utils import run_bass_kernel_spmd

B = 8388608
N_CORES = 8
PER_CORE = B // N_CORES  # 1048576
P = 128
FD_TOTAL = PER_CORE // P  # 8192 per array per core

# Per-chunk free-dim sizes (per array). Even indices go to the SP ring,
# odd to the ACT ring; each ring carries 4096 (4 MiB of pair data).
CHUNK_FDS = [2048, 2048, 1024, 1024, 512, 512, 512, 512]
assert sum(CHUNK_FDS) == FD_TOTAL
N_CHUNKS = len(CHUNK_FDS)
CHUNK_OFFS = np.concatenate([[0], np.cumsum(CHUNK_FDS)]).tolist()

F32 = mybir.dt.float32

_NC = None


def build_nc():
    nc = bass.Bass()
    pairs = nc.dram_tensor("pairs", [2 * PER_CORE], F32, kind="ExternalInput")
    out = nc.dram_tensor("out", [P, N_CHUNKS], F32, kind="ExternalOutput")

    with ExitStack() as ctx:
        pair_buf = ctx.enter_context(
            nc.sbuf_tensor("pair_buf", [P, 2 * FD_TOTAL], F32)
        )
        mask_buf = ctx.enter_context(nc.sbuf_tensor("mask_buf", [P, FD_TOTAL], F32))
        partials = ctx.enter_context(nc.sbuf_tensor("partials", [P, N_CHUNKS], F32))
        chunk_sems = [
            ctx.enter_context(nc.semaphore(f"chunk{k}")) for k in range(N_CHUNKS)
        ]
        dve_sem = ctx.enter_context(nc.semaphore("dve_sem"))
        out_sem = ctx.enter_context(nc.semaphore("out_sem"))
        block = ctx.enter_context(nc.Block())

        def chunk_dma(eng, k):
            fd = CHUNK_FDS[k]
            off = CHUNK_OFFS[k]
            src = bass.AP(pairs, 2 * P * off, [[2 * fd, P], [1, 2 * fd]])
            dst = pair_buf[:, 2 * off : 2 * (off + fd)]
            eng.dma_start(dst, src).then_inc(chunk_sems[k], 16)

        @block.sync
        def _(sync: bass.BassEngine):
            for k in range(0, N_CHUNKS, 2):
                chunk_dma(sync, k)
            sync.wait_ge(dve_sem, N_CHUNKS)
            sync.dma_start(out[:], partials[:]).then_inc(out_sem, 16)

        @block.scalar
        def _(scalar: bass.BassEngine):
            for k in range(1, N_CHUNKS, 2):
                chunk_dma(scalar, k)

        @block.vector
        def _(vector: bass.BassEngine):
            for k in range(N_CHUNKS):
                fd = CHUNK_FDS[k]
                off = CHUNK_OFFS[k]
                vector.wait_ge(chunk_sems[k], 16)
                # mask = (ant bypass 0.0) is_gt syn -> 1.0/0.0
                # partials[:, k] = free-dim sum of mask (same instruction)
                vector.scalar_tensor_tensor(
                    out=mask_buf[:, off : off + fd],
                    in0=pair_buf[:, 2 * off : 2 * off + fd],
                    scalar=0.0,
                    in1=pair_buf[:, 2 * off + fd : 2 * (off + fd)],
                    op0=mybir.AluOpType.bypass,
                    op1=mybir.AluOpType.is_gt,
                    accum_out=partials[:, k : k + 1],
                ).then_inc(dve_sem, 1)

    return nc


def _make_pairs(synonymy_score, antonymy_score):
    """Build the per-core flat pair tensor: consecutive [128, 2, fd_j]
    blocks with ant rows first (in0), then syn rows (in1)."""
    syn = np.asarray(synonymy_score, dtype=np.float32).reshape(
        N_CORES, P, FD_TOTAL
    )
    ant = np.asarray(antonymy_score, dtype=np.float32).reshape(
        N_CORES, P, FD_TOTAL
    )
    blocks = []
    for k in range(N_CHUNKS):
        s, e = CHUNK_OFFS[k], CHUNK_OFFS[k + 1]
        blk = np.stack([ant[:, :, s:e], syn[:, :, s:e]], axis=2)  # [C,P,2,fd]
        blocks.append(blk.reshape(N_CORES, -1))
    return np.ascontiguousarray(np.concatenate(blocks, axis=1))  # [C, 2*PER_CORE]


def run(inputs, trace=False, trace_cores=None):
    """Run the SPMD kernel on 8 cores. Returns (result_scalar, BassKernelResults)."""
    global _NC
    if _NC is None:
        _NC = build_nc()

    pairs = _make_pairs(inputs["synonymy_score"], inputs["antonymy_score"])
    in_maps = [{"pairs": pairs[c]} for c in range(N_CORES)]
    bkr = run_bass_kernel_spmd(
        _NC,
        in_maps,
        list(range(N_CORES)),
        trace=trace,
        trace_cores=trace_cores,
    )
    total = sum(
        np.asarray(r["out"], dtype=np.float64).sum() for r in bkr.results
    )
    result = np.float32(-(total / B))
    return result, bkr


def kernel(S1_out, synonymy_score, antonymy_score):
    result, _ = run(
        {"synonymy_score": synonymy_score, "antonymy_score": antonymy_score}
    )
    return result


# revision 15
# speedup vs baseline: 2.2369x; 1.6313x over previous
"""Trainium2 Bass kernel for -mean(antonymy_score > synonymy_score).

Strategy: pure data-parallel over 8 NeuronCores. Each core receives a
contiguous 1/8 slice of the batch. On the host, antonymy/synonymy are
interleaved into one flat "pairs" tensor laid out as consecutive
[128, 2, fd_j] chunk blocks so each chunk arrives in a single contiguous
DMA carrying one semaphore (the walrus codegen path here allows only one
sync wait per instruction). Chunks alternate between the two HWDGE rings
(SP + ACT) so the two streams transfer concurrently (~341 GB/s observed,
~358 GB/s per-core HBM cap). Chunk sizes taper (2MB, 1MB, 0.5MB, 0.5MB
per ring) so the DVE work left after the last chunk lands is minimal.

Each chunk is consumed by one fused DVE scalar_tensor_tensor:
mask = (ant bypass 0) is_gt syn, with accum_out producing the
per-partition free-dim sum in the same pass. The [128, n_chunks] partial
counts are DMA'd back and the host computes -total/B (exact:
integer-valued fp32 counts). S1_out is unused by the computation - it
only fixes the batch size.

Raw Bass (no TileContext) keeps the program pre/postamble to a single
all-engine barrier instead of Tile's ~10us of drains + EVSEM
butterflies. One semaphore per chunk DMA: two in-flight DMAs sharing a
sem can interleave their 16 per-SDMA-engine increments, so a cumulative
wait could fire with the earlier chunk still incomplete.
"""

from contextlib import ExitStack

import numpy as np

import concourse.bass as bass
import concourse.mybir as mybir
from concourse.bass_utils import run_bass_kernel_spmd

B = 8388608
N_CORES = 8
PER_CORE = B // N_CORES  # 1048576
P = 128
FD_TOTAL = PER_CORE // P  # 8192 per array per core

# Per-chunk free-dim sizes (per array). Even indices go to the SP ring,
# odd to the ACT ring. The rings are deliberately unbalanced (SP carries
# 3968, ACT 4224) so the SP ring finishes early and only the ACT ring's
# small final chunk leaves DVE work exposed after the stream ends.
CHUNK_FDS = [2048, 2048, 1024, 1024, 512, 768, 384, 384]
assert sum(CHUNK_FDS) == FD_TOTAL
N_CHUNKS = len(CHUNK_FDS)
CHUNK_OFFS = np.concatenate([[0], np.cumsum(CHUNK_FDS)]).tolist()

F32 = mybir.dt.float32

_NC = None


def build_nc():
    nc = bass.Bass()
    pairs = nc.dram_tensor("pairs", [2 * PER_CORE], F32, kind="ExternalInput")
    out = nc.dram_tensor("out", [P, N_CHUNKS], F32, kind="ExternalOutput")

    with ExitStack() as ctx:
        pair_buf = ctx.enter_context(
            nc.sbuf_tensor("pair_buf", [P, 2 * FD_TOTAL], F32)
        )
        mask_buf = ctx.enter_context(nc.sbuf_tensor("mask_buf", [P, FD_TOTAL], F32))
        partials = ctx.enter_context(nc.sbuf_tensor("partials", [P, N_CHUNKS], F32))
        chunk_sems = [
            ctx.enter_context(nc.semaphore(f"chunk{k}")) for k in range(N_CHUNKS)
        ]
        dve_sem = ctx.enter_context(nc.semaphore("dve_sem"))
        out_sem = ctx.enter_context(nc.semaphore("out_sem"))
        block = ctx.enter_context(nc.Block())

        def chunk_dma(eng, k):
            fd = CHUNK_FDS[k]
            off = CHUNK_OFFS[k]
            src = bass.AP(pairs, 2 * P * off, [[2 * fd, P], [1, 2 * fd]])
            dst = pair_buf[:, 2 * off : 2 * (off + fd)]
            eng.dma_start(dst, src).then_inc(chunk_sems[k], 16)

        @block.sync
        def _(sync: bass.BassEngine):
            for k in range(0, N_CHUNKS, 2):
                chunk_dma(sync, k)
            sync.wait_ge(dve_sem, N_CHUNKS)
            sync.dma_start(out[:], partials[:]).then_inc(out_sem, 16)

        @block.scalar
        def _(scalar: bass.BassEngine):
            for k in range(1, N_CHUNKS, 2):
                chunk_dma(scalar, k)

        @block.vector
        def _(vector: bass.BassEngine):
            for k in range(N_CHUNKS):
                fd = CHUNK_FDS[k]
                off = CHUNK_OFFS[k]
                vector.wait_ge(chunk_sems[k], 16)
                # mask = (ant bypass 0.0) is_gt syn -> 1.0/0.0
                # partials[:, k] = free-dim sum of mask (same instruction)
                vector.scalar_tensor_tensor(
                    out=mask_buf[:, off : off + fd],
                    in0=pair_buf[:, 2 * off : 2 * off + fd],
                    scalar=0.0,
                    in1=pair_buf[:, 2 * off + fd : 2 * (off + fd)],
                    op0=mybir.AluOpType.bypass,
                    op1=mybir.AluOpType.is_gt,
                    accum_out=partials[:, k : k + 1],
                ).then_inc(dve_sem, 1)

    _strip_const_preamble(nc)
    return nc


def _strip_const_preamble(nc):
    """Bass.__init__ unconditionally materializes four const SBUF tensors
    (gpsimd memsets) and an all-engine barrier before main. This kernel
    reads none of the consts and has no cross-engine dependency at start
    (every data hand-off goes through explicit semaphores), so drop them
    from the entry block - worth ~0.8us of program preamble."""
    main_bb = nc.main_func.blocks[0]
    assert main_bb.name == "main"

    def removable(ins):
        t = type(ins).__name__
        if t == "InstMemset":
            return getattr(ins.outs[0], "memref", "").startswith("const-")
        return t in ("InstDrain", "InstEventSemaphore")

    main_bb.instructions[:] = [
        ins for ins in main_bb.instructions if not removable(ins)
    ]


def _make_pairs(synonymy_score, antonymy_score):
    """Build the per-core flat pair tensor: consecutive [128, 2, fd_j]
    blocks with ant rows first (in0), then syn rows (in1)."""
    syn = np.asarray(synonymy_score, dtype=np.float32).reshape(
        N_CORES, P, FD_TOTAL
    )
    ant = np.asarray(antonymy_score, dtype=np.float32).reshape(
        N_CORES, P, FD_TOTAL
    )
    blocks = []
    for k in range(N_CHUNKS):
        s, e = CHUNK_OFFS[k], CHUNK_OFFS[k + 1]
        blk = np.stack([ant[:, :, s:e], syn[:, :, s:e]], axis=2)  # [C,P,2,fd]
        blocks.append(blk.reshape(N_CORES, -1))
    return np.ascontiguousarray(np.concatenate(blocks, axis=1))  # [C, 2*PER_CORE]


def run(inputs, trace=False, trace_cores=None):
    """Run the SPMD kernel on 8 cores. Returns (result_scalar, BassKernelResults)."""
    global _NC
    if _NC is None:
        _NC = build_nc()

    pairs = _make_pairs(inputs["synonymy_score"], inputs["antonymy_score"])
    in_maps = [{"pairs": pairs[c]} for c in range(N_CORES)]
    bkr = run_bass_kernel_spmd(
        _NC,
        in_maps,
        list(range(N_CORES)),
        trace=trace,
        trace_cores=trace_cores,
    )
    total = sum(
        np.asarray(r["out"], dtype=np.float64).sum() for r in bkr.results
    )
    result = np.float32(-(total / B))
    return result, bkr


def kernel(S1_out, synonymy_score, antonymy_score):
    result, _ = run(
        {"synonymy_score": synonymy_score, "antonymy_score": antonymy_score}
    )
    return result
